# revision 1
# baseline (speedup 1.0000x reference)
"""Segment-mean pooling kernel for Trainium2 (8 NeuronCores, data-parallel).

Input : emb_vector [1024, 2048, 64] f32
Output: [1024, 32, 64] f32 — mean over 32 ragged field segments
        (sizes [32, 64, 96, 64] * 8, summing to 2048).

Sharding: batch axis 0 split across 8 cores (128 rows each). Per core the
128 batch rows sit on the 128 SBUF partitions; fields*embed is the free
axis. The segment pattern repeats every 256 fields, so each core streams 8
groups of [128, 256*64] f32 (64 KiB/partition, contiguous in DRAM; 8 MiB
per DMA, double-buffered -> DMA runs at the ~358 GB/s HBM-per-core limit).

Per group ('mix_sr'): DVE reduces segments 0-2 straight off the raw tile
with strided XY-reduces (~0.67 elem/cycle demand on one SBUF read port, no
intermediate writes) while GPSIMD folds segment 3's two 32-field blocks
with contiguous in-place pairwise tensor_adds; the last group instead uses
a 6/2 DVE/pool balanced fold so the kernel tail isn't gated by one engine.
This split keeps both compute engines well under the DMA span (DVE ~154us,
pool ~66us vs DMA ~189us per full pass) while minimizing SBUF port-ops,
which is what contends with the DMA write stream (measured: quiet-window
round minima sit at the pure-DMA floor). Scale-by-1/size and the output
DMA issue from the ACT engine so the SP sequencer's HWDGE ring only ever
streams input loads (out-DMA sem-waits on SP bubble the input stream;
measured ~+18 us). The Tile cost-model TimelineSim confirms the schedule
is bubble-free: simulated marginal 189.3 us/rep = the DMA floor.

Measured marginal per-execution time: ~210-230 us on a quiet device vs a
~188 us pure-DMA floor (65 MiB/core at the HBM limit); device-sharing
bursts inflate both.
"""

import os
import sys
from functools import lru_cache

import numpy as np

for _p in ("/opt/trn_rl_repo", os.path.expanduser("~/.axon_site/_ro/trn_rl_repo")):
    if os.path.isdir(_p) and _p not in sys.path:
        sys.path.insert(0, _p)

import concourse.bass as bass
import concourse.bacc as bacc
import concourse.mybir as mybir
from concourse import tile

N_CORES = 8
BATCH, FIELDS, D = 1024, 2048, 64
B_LOC = BATCH // N_CORES          # 128 batch rows per core = SBUF partitions
GROUP_F = 256                     # fields per repeating segment group
GROUPS = FIELDS // GROUP_F        # 8
SEG_OFF = (0, 32, 96, 192)        # field offsets within a group
SEG_SZ = (32, 64, 96, 64)         # segment sizes
NSEG_G = 4                        # segments per group
NSEG = NSEG_G * GROUPS            # 32
FP32 = mybir.dt.float32


def _emit_group(nc, t, o, variant: str, nk_override: int | None = None):
    """Reduce one group tile t [128, 256*64] into segment means o [128, 4*64].

    variant 'strided': 4 strided-X vector reduces (v1).
    variant 'tree': in-place contiguous pairwise fold — every segment is a
    multiple of 32 fields, so fold each 32-field block down to one 64-wide
    block sum (contiguous TT adds run at 1 elem/cycle vs ~1.5 for strided
    reduce), then combine blocks per segment with small strided reduces.
    """
    BLK = 32 * D  # one folded 32-field block: 2048 elems
    if variant == "strided":
        t3 = t[:].rearrange("b (f d) -> b d f", d=D)
        for si in range(NSEG_G):
            f0, sz = SEG_OFF[si], SEG_SZ[si]
            nc.vector.reduce_sum(
                out=o[:, si * D : (si + 1) * D],
                in_=t3[:, :, f0 : f0 + sz],
                axis=mybir.AxisListType.X,
            )
            nc.scalar.mul(
                out=o[:, si * D : (si + 1) * D],
                in_=o[:, si * D : (si + 1) * D],
                mul=1.0 / sz,
            )
        return

    if variant in ("tree", "tree_gps", "tree_gps3", "tree_gps4",
                   "tree_gps5"):
        # view [b, blk, within]: fold `within` 1024->512->...->64 in place.
        # tree_gps: blocks 6-7 (segment 3) fold on GPSIMD instead of DVE;
        # tree_gps3 moves block 5 (last third of segment 2) there as well.
        nk = {"tree": 8, "tree_gps": 6, "tree_gps3": 5, "tree_gps4": 4,
              "tree_gps5": 3}[variant]
        if nk_override is not None:
            nk = nk_override
        for width in (1024, 512, 256, 128, 64):
            v = t[:].rearrange("b (k w) -> b k w", w=BLK)
            nc.vector.tensor_add(
                v[:, :nk, :width], v[:, :nk, :width],
                v[:, :nk, width : 2 * width],
            )
            if nk < 8:
                nc.gpsimd.tensor_add(
                    v[:, nk:, :width], v[:, nk:, :width],
                    v[:, nk:, width : 2 * width],
                )
        if nk < 8:
            o3 = o[:, 3 * D : 4 * D]
            nc.gpsimd.tensor_add(
                o3, t[:, 6 * BLK : 6 * BLK + D], t[:, 7 * BLK : 7 * BLK + D]
            )
            nc.gpsimd.tensor_scalar_mul(o3, o3, 1.0 / SEG_SZ[3])
        # block sums now at t[:, k*BLK : k*BLK + 64] for k in 0..7
        blocks = t[:].rearrange("b (k w) -> b w k", w=BLK)[:, :D, :]
        seg_blocks = ((0, 1), (1, 3), (3, 6), (6, 8))
        for si, (k0, k1) in enumerate(seg_blocks):
            if variant.startswith("tree_gps") and si == 3:
                continue  # handled on GPSIMD above
            osl = o[:, si * D : (si + 1) * D]
            if k1 - k0 == 1:
                nc.scalar.activation(
                    out=osl,
                    in_=t[:, k0 * BLK : k0 * BLK + D],
                    func=mybir.ActivationFunctionType.Copy,
                    scale=1.0 / SEG_SZ[si],
                )
            else:
                nc.vector.reduce_sum(
                    out=osl, in_=blocks[:, :, k0:k1], axis=mybir.AxisListType.X
                )
                nc.scalar.mul(out=osl, in_=osl, mul=1.0 / SEG_SZ[si])
        return

    if variant == "mix_sr":
        # Port-minimal mix: DVE reduces segments 0-2 straight off the raw
        # tile with strided XY-reduces (1 read port, ~0.67 elem/cycle, no
        # intermediate writes); pool folds segment 3's two blocks. About
        # half the SBUF port-ops of the 4/4 fold split.
        t4 = t[:].rearrange("b (k f d) -> b d k f", k=8, d=D)
        for si, (k0, k1) in enumerate(((0, 1), (1, 3), (3, 6))):
            osl = o[:, si * D : (si + 1) * D]
            nc.vector.reduce_sum(
                out=osl, in_=t4[:, :, k0:k1, :], axis=mybir.AxisListType.XY
            )
            nc.scalar.mul(out=osl, in_=osl, mul=1.0 / SEG_SZ[si])
        for width in (1024, 512, 256, 128, 64):
            v = t[:].rearrange("b (k w) -> b k w", w=BLK)
            nc.gpsimd.tensor_add(
                v[:, 6:, :width], v[:, 6:, :width],
                v[:, 6:, width : 2 * width],
            )
        o3 = o[:, 3 * D : 4 * D]
        nc.gpsimd.tensor_add(
            o3, t[:, 6 * BLK : 6 * BLK + D], t[:, 7 * BLK : 7 * BLK + D]
        )
        nc.gpsimd.tensor_scalar_mul(o3, o3, 1.0 / SEG_SZ[3])
        return

    assert variant == "hybrid"
    # One contiguous in-place fold level (each 32-field block: fields
    # [0:16) += [16:32)), then one strided XY-reduce per segment over the
    # folded fields of its blocks.
    v = t[:].rearrange("b (k w) -> b k w", w=BLK)
    nc.vector.tensor_add(v[:, :, :1024], v[:, :, :1024], v[:, :, 1024:2048])
    # folded tile view [b, k, f(16), d] -> reduce per segment over (k, f)
    t4 = t[:].rearrange("b (k f d) -> b d k f", k=8, d=D)  # [b, d, k, f16]
    seg_blocks = ((0, 1), (1, 3), (3, 6), (6, 8))
    for si, (k0, k1) in enumerate(seg_blocks):
        osl = o[:, si * D : (si + 1) * D]
        nc.vector.reduce_sum(
            out=osl,
            in_=t4[:, :, k0:k1, :16],
            axis=mybir.AxisListType.XY,
        )
        nc.scalar.mul(out=osl, in_=osl, mul=1.0 / SEG_SZ[si])


@lru_cache(maxsize=16)
def _build(reps: int = 1, variant: str = "mix_sr", chunk_f: int = 256,
           bufs: int = 2, out_eng: str = "scalar"):
    """reps>1 repeats the whole workload back-to-back inside one NEFF —
    used only for timing (marginal per-rep time cancels dispatch+preamble
    overheads)."""
    nc = bacc.Bacc(
        "TRN2", target_bir_lowering=False, debug=False, num_devices=N_CORES
    )
    x = nc.declare_dram_parameter("x", [B_LOC, FIELDS, D], FP32, isOutput=False)
    y = nc.declare_dram_parameter("y", [B_LOC, NSEG, D], FP32, isOutput=True)
    xf = x.rearrange("b f d -> b (f d)")

    with tile.TileContext(nc) as tc:
        with (
            tc.tile_pool(name="inp", bufs=bufs) as inp_pool,
            tc.tile_pool(name="outp", bufs=2) as out_pool,
            tc.tile_pool(name="tmpp", bufs=2) as tmp_pool,
        ):
            for _ in range(reps):
                if chunk_f == GROUP_F:
                    o_all = None
                    if out_eng == "final":
                        o_all = out_pool.tile([B_LOC, NSEG * D], FP32,
                                              tag="oall")
                    for g in range(GROUPS):
                        t = inp_pool.tile(
                            [B_LOC, GROUP_F * D], FP32, tag="in"
                        )
                        nc.sync.dma_start(
                            out=t[:],
                            in_=xf[:, g * GROUP_F * D : (g + 1) * GROUP_F * D],
                        )
                        # last group: rebalance toward a 6/2 DVE/pool fold so
                        # the kernel tail isn't gated by one slow engine chain
                        g_variant, nk_last = variant, None
                        if g == GROUPS - 1 and (
                            variant.startswith("tree_gps")
                            or variant == "mix_sr"
                        ):
                            g_variant, nk_last = "tree_gps", 6
                        if out_eng == "final":
                            o = o_all[:, g * NSEG_G * D : (g + 1) * NSEG_G * D]
                            _emit_group(nc, t, o, g_variant, nk_last)
                        else:
                            o = out_pool.tile([B_LOC, NSEG_G * D], FP32,
                                              tag="out")
                            _emit_group(nc, t, o[:], g_variant, nk_last)
                            dma_eng = {
                                "sync": nc.sync,
                                "gpsimd": nc.gpsimd,
                                "scalar": nc.scalar,
                            }[out_eng]
                            dma_eng.dma_start(
                                out=y[:, g * NSEG_G : (g + 1) * NSEG_G, :],
                                in_=o[:].rearrange("b (s d) -> b s d", d=D),
                            )
                    if out_eng == "final":
                        nc.scalar.dma_start(
                            out=y[:, :, :],
                            in_=o_all[:].rearrange("b (s d) -> b s d", d=D),
                        )
                else:
                    assert chunk_f == GROUP_F // 2 and variant == "strided"
                    HF = chunk_f * D  # 8192
                    for g in range(GROUPS):
                        o = out_pool.tile([B_LOC, NSEG_G * D], FP32, tag="out")
                        for h in range(2):
                            t = inp_pool.tile([B_LOC, HF], FP32, tag="in")
                            nc.sync.dma_start(
                                out=t[:],
                                in_=xf[
                                    :,
                                    (2 * g + h) * HF : (2 * g + h + 1) * HF,
                                ],
                            )
                            t3 = t[:].rearrange("b (f d) -> b d f", d=D)
                            if h == 0:
                                # fields 0:128 = seg0(32), seg1(64), seg2a(32)
                                nc.vector.reduce_sum(
                                    out=o[:, 0:D], in_=t3[:, :, 0:32],
                                    axis=mybir.AxisListType.X,
                                )
                                nc.vector.reduce_sum(
                                    out=o[:, D : 2 * D], in_=t3[:, :, 32:96],
                                    axis=mybir.AxisListType.X,
                                )
                                nc.vector.reduce_sum(
                                    out=o[:, 2 * D : 3 * D],
                                    in_=t3[:, :, 96:128],
                                    axis=mybir.AxisListType.X,
                                )
                            else:
                                # fields 128:256 = seg2b(64), seg3(64)
                                tmp = tmp_pool.tile([B_LOC, D], FP32, tag="t2")
                                nc.vector.reduce_sum(
                                    out=tmp[:], in_=t3[:, :, 0:64],
                                    axis=mybir.AxisListType.X,
                                )
                                nc.vector.tensor_add(
                                    o[:, 2 * D : 3 * D], o[:, 2 * D : 3 * D],
                                    tmp[:],
                                )
                                nc.vector.reduce_sum(
                                    out=o[:, 3 * D : 4 * D],
                                    in_=t3[:, :, 64:128],
                                    axis=mybir.AxisListType.X,
                                )
                        for si in range(NSEG_G):
                            nc.scalar.mul(
                                out=o[:, si * D : (si + 1) * D],
                                in_=o[:, si * D : (si + 1) * D],
                                mul=1.0 / SEG_SZ[si],
                            )
                        dma_eng = nc.sync if out_eng == "sync" else nc.gpsimd
                        dma_eng.dma_start(
                            out=y[:, g * NSEG_G : (g + 1) * NSEG_G, :],
                            in_=o[:].rearrange("b (s d) -> b s d", d=D),
                        )
    nc.finalize()
    return nc


def _sharded_from_nc(nc):
    """Build the 8-way-sharded jitted executable for a finalized Bass module.

    Mirrors bass2jax.run_bass_via_pjrt's multi-core branch (shard_map over a
    'core' mesh; per-device shard == the BIR-declared per-core shape) but
    without output-buffer donation so the same function can be called in a
    timing loop with device-resident inputs.
    """
    import jax
    from jax.experimental.shard_map import shard_map
    from jax.sharding import Mesh, NamedSharding, PartitionSpec

    from concourse import bass2jax, mybir as _mybir

    bass2jax.install_neuronx_cc_hook()

    in_names, out_names, out_avals, zero_outs = [], [], [], []
    partition_name = (
        nc.partition_id_tensor.name if nc.partition_id_tensor else None
    )
    for alloc in nc.m.functions[0].allocations:
        if not isinstance(alloc, _mybir.MemoryLocationSet):
            continue
        name = alloc.memorylocations[0].name
        if alloc.kind == "ExternalInput":
            if name != partition_name:
                in_names.append(name)
        elif alloc.kind == "ExternalOutput":
            shape = tuple(alloc.tensor_shape)
            dtype = _mybir.dt.np(alloc.dtype)
            out_names.append(name)
            out_avals.append(jax.core.ShapedArray(shape, dtype))
            zero_outs.append(np.zeros(shape, dtype))
    n_params = len(in_names)
    all_in_names = list(in_names) + list(out_names)
    if partition_name is not None:
        all_in_names.append(partition_name)

    def _body(*args):
        operands = list(args)
        if partition_name is not None:
            operands.append(bass2jax.partition_id_tensor())
        outs = bass2jax._bass_exec_p.bind(
            *operands,
            out_avals=tuple(out_avals),
            in_names=tuple(all_in_names),
            out_names=tuple(out_names),
            lowering_input_output_aliases=(),
            sim_require_finite=True,
            sim_require_nnan=True,
            nc=nc,
        )
        return tuple(outs)

    devices = jax.devices()[:N_CORES]
    mesh = Mesh(np.asarray(devices), ("core",))
    n_outs = len(out_names)
    in_specs = (PartitionSpec("core"),) * (n_params + n_outs)
    out_specs = (PartitionSpec("core"),) * n_outs
    sharded = jax.jit(
        shard_map(
            _body, mesh=mesh, in_specs=in_specs, out_specs=out_specs,
            check_rep=False,
        ),
        keep_unused=True,
    )
    in_sharding = NamedSharding(mesh, PartitionSpec("core"))
    return sharded, zero_outs, in_sharding


@lru_cache(maxsize=4)
def _compiled(reps: int = 1):
    return _sharded_from_nc(_build(reps))


def _put_inputs(emb_vector: np.ndarray, reps: int = 1):
    import jax

    sharded, zero_outs, in_sharding = _compiled(reps)
    x = np.ascontiguousarray(emb_vector, dtype=np.float32)
    dx = jax.device_put(x, in_sharding)
    dzeros = [
        jax.device_put(
            np.zeros((N_CORES * z.shape[0], *z.shape[1:]), z.dtype), in_sharding
        )
        for z in zero_outs
    ]
    return sharded, dx, dzeros


def kernel(emb_vector: np.ndarray) -> np.ndarray:
    sharded, dx, dzeros = _put_inputs(emb_vector)
    (out,) = sharded(dx, *dzeros)
    return np.asarray(out)


def bench(emb_vector: np.ndarray, iters: int = 30, warmup: int = 5,
          reps: int = 1):
    """Steady-state per-call wall time of the sharded executable, ns."""
    import time

    sharded, dx, dzeros = _put_inputs(emb_vector, reps)
    for _ in range(warmup):
        (out,) = sharded(dx, *dzeros)
    out.block_until_ready()
    t0 = time.perf_counter()
    for _ in range(iters):
        (out,) = sharded(dx, *dzeros)
    out.block_until_ready()
    t1 = time.perf_counter()
    return (t1 - t0) / iters * 1e9, np.asarray(out)


def measure_exec_ns(emb_vector: np.ndarray, lo: int = 4, hi: int = 12,
                    iters: int = 20, n_pairs: int = 7):
    """Marginal per-execution HW time via in-NEFF workload repetition:
    (t(hi reps) - t(lo reps)) / (hi - lo) cancels per-dispatch client/RPC
    overhead and NEFF preamble/postamble. hi/lo timing loops are
    interleaved (median of per-pair diffs) so device-load drift cancels."""
    import time

    sharded_hi, dx, dz_hi = _put_inputs(emb_vector, hi)
    sharded_lo, _, dz_lo = _put_inputs(emb_vector, lo)
    for _ in range(4):
        (out,) = sharded_hi(dx, *dz_hi)
        (out_lo,) = sharded_lo(dx, *dz_lo)
    out.block_until_ready()
    out_lo.block_until_ready()
    diffs = []
    for _ in range(n_pairs):
        t0 = time.perf_counter()
        for _ in range(iters):
            (out,) = sharded_hi(dx, *dz_hi)
        out.block_until_ready()
        t1 = time.perf_counter()
        for _ in range(iters):
            (out_lo,) = sharded_lo(dx, *dz_lo)
        out_lo.block_until_ready()
        t2 = time.perf_counter()
        diffs.append(((t1 - t0) - (t2 - t1)) / iters * 1e9)
    med = sorted(diffs)[len(diffs) // 2]
    return med / (hi - lo), np.asarray(out)



# revision 3
# speedup vs baseline: 1.4541x; 1.4541x over previous
"""Segment-mean pooling kernel for Trainium2 (8 NeuronCores, data-parallel).

Input : emb_vector [1024, 2048, 64] f32
Output: [1024, 32, 64] f32 — mean over 32 ragged field segments
        (sizes [32, 64, 96, 64] * 8, summing to 2048).

Sharding: batch axis 0 split across 8 cores (128 rows each). Per core the
128 batch rows sit on the 128 SBUF partitions; fields*embed is the free
axis. The segment pattern repeats every 256 fields, so each core streams 8
groups of [128, 256*64] f32 (64 KiB/partition, contiguous in DRAM).

DMA structure (the lever that matters): each group loads as TWO 4-MiB
dma_starts into one 64-KiB/partition tile, with a 3-deep tile ring — up to
six 4-MiB input DMAs (24 MiB) queued on the SP HWDGE ring at all times.
Chip HBM bandwidth is dynamically shared across the 8 NeuronCores, so a
core only sustains its full share when its DMA queue never drains; the
shallow 2-deep/8-MiB schedule measured 250-260 us/rep marginal while this
one measures 185-220 us/rep (quiet-window min ~175 us ~= the 2.9 TB/s
chip-aggregate HBM roofline; device-sharing bursts inflate the median).
A/B evidence (interleaved hi/lo-reps marginal diffs, 16-24 pairs): deeper
+ finer beats wider on every statistic; single-ring input loads beat
alternating sync/scalar rings; a 16x4-MiB flat chunk ring and per-128-field
compute (q6) land between.

Per group ('mix_sr' split, chosen to keep both compute engines off each
other's SBUF ports): DVE reduces segments 0-2 straight off the raw tile
with strided XY-reduces (1 read port, no intermediate writes) while GPSIMD
folds segment 3's two 32-field blocks with contiguous in-place pairwise
tensor_adds; the last group instead uses a 6/2 DVE/pool balanced fold so
the kernel tail isn't gated by one engine chain. Scale-by-1/size and the
output DMA issue from the ACT engine so the SP sequencer's HWDGE ring only
ever streams input loads. Compute totals (DVE ~155 us, pool ~66 us per
pass) sit well under the DMA span, so the schedule stays DMA-bound.
"""

import os
import sys
from functools import lru_cache

import numpy as np

for _p in ("/opt/trn_rl_repo", os.path.expanduser("~/.axon_site/_ro/trn_rl_repo")):
    if os.path.isdir(_p) and _p not in sys.path:
        sys.path.insert(0, _p)

import concourse.bass as bass
import concourse.bacc as bacc
import concourse.mybir as mybir
from concourse import tile

N_CORES = 8
BATCH, FIELDS, D = 1024, 2048, 64
B_LOC = BATCH // N_CORES          # 128 batch rows per core = SBUF partitions
GROUP_F = 256                     # fields per repeating segment group
GROUPS = FIELDS // GROUP_F        # 8
SEG_SZ = (32, 64, 96, 64)         # segment sizes (fields)
SEG_BLOCKS = ((0, 1), (1, 3), (3, 6), (6, 8))  # 32-field blocks per segment
NSEG_G = 4                        # segments per group
NSEG = NSEG_G * GROUPS            # 32
FP32 = mybir.dt.float32
BLK = 32 * D                      # one 32-field block: 2048 elems


def _emit_group_mix(nc, t, o):
    """Segment means for one group tile t [128, 256*64] into o [128, 4*64].

    DVE strided XY-reduces segments 0-2 off the raw tile (single read
    port); GPSIMD folds segment 3's two blocks pairwise in place.
    """
    t4 = t[:].rearrange("b (k f d) -> b d k f", k=8, d=D)
    for si in range(3):
        k0, k1 = SEG_BLOCKS[si]
        osl = o[:, si * D : (si + 1) * D]
        nc.vector.reduce_sum(
            out=osl, in_=t4[:, :, k0:k1, :], axis=mybir.AxisListType.XY
        )
        nc.scalar.mul(out=osl, in_=osl, mul=1.0 / SEG_SZ[si])
    for width in (1024, 512, 256, 128, 64):
        v = t[:].rearrange("b (k w) -> b k w", w=BLK)
        nc.gpsimd.tensor_add(
            v[:, 6:, :width], v[:, 6:, :width], v[:, 6:, width : 2 * width]
        )
    o3 = o[:, 3 * D : 4 * D]
    nc.gpsimd.tensor_add(
        o3, t[:, 6 * BLK : 6 * BLK + D], t[:, 7 * BLK : 7 * BLK + D]
    )
    nc.gpsimd.tensor_scalar_mul(o3, o3, 1.0 / SEG_SZ[3])


def _emit_group_tail(nc, t, o):
    """Last group: 6/2 DVE/pool pairwise fold so the tail isn't gated by
    one slow engine chain (DVE folds blocks 0-5, GPSIMD blocks 6-7)."""
    nk = 6
    for width in (1024, 512, 256, 128, 64):
        v = t[:].rearrange("b (k w) -> b k w", w=BLK)
        nc.vector.tensor_add(
            v[:, :nk, :width], v[:, :nk, :width], v[:, :nk, width : 2 * width]
        )
        nc.gpsimd.tensor_add(
            v[:, nk:, :width], v[:, nk:, :width], v[:, nk:, width : 2 * width]
        )
    o3 = o[:, 3 * D : 4 * D]
    nc.gpsimd.tensor_add(
        o3, t[:, 6 * BLK : 6 * BLK + D], t[:, 7 * BLK : 7 * BLK + D]
    )
    nc.gpsimd.tensor_scalar_mul(o3, o3, 1.0 / SEG_SZ[3])
    blocks = t[:].rearrange("b (k w) -> b w k", w=BLK)[:, :D, :]
    for si in range(3):
        k0, k1 = SEG_BLOCKS[si]
        osl = o[:, si * D : (si + 1) * D]
        if k1 - k0 == 1:
            nc.scalar.activation(
                out=osl,
                in_=t[:, k0 * BLK : k0 * BLK + D],
                func=mybir.ActivationFunctionType.Copy,
                scale=1.0 / SEG_SZ[si],
            )
        else:
            nc.vector.reduce_sum(
                out=osl, in_=blocks[:, :, k0:k1], axis=mybir.AxisListType.X
            )
            nc.scalar.mul(out=osl, in_=osl, mul=1.0 / SEG_SZ[si])


@lru_cache(maxsize=16)
def _build(reps: int = 1):
    """reps>1 repeats the whole workload back-to-back inside one NEFF —
    used only for timing (marginal per-rep time cancels dispatch+preamble
    overheads)."""
    nc = bacc.Bacc(
        "TRN2", target_bir_lowering=False, debug=False, num_devices=N_CORES
    )
    x = nc.declare_dram_parameter("x", [B_LOC, FIELDS, D], FP32, isOutput=False)
    y = nc.declare_dram_parameter("y", [B_LOC, NSEG, D], FP32, isOutput=True)
    xf = x.rearrange("b f d -> b (f d)")

    GW = GROUP_F * D      # group width in elems (64 KiB/partition)
    QW = GW // 4          # quarter-group: one 2-MiB DMA

    with tile.TileContext(nc) as tc:
        with (
            tc.tile_pool(name="inp", bufs=3) as inp_pool,
            tc.tile_pool(name="outp", bufs=3) as out_pool,
        ):
            for _ in range(reps):
                for g in range(GROUPS):
                    t = inp_pool.tile([B_LOC, GW], FP32, tag="in")
                    for h in range(4):
                        nc.sync.dma_start(
                            out=t[:, h * QW : (h + 1) * QW],
                            in_=xf[:, g * GW + h * QW : g * GW + (h + 1) * QW],
                        )
                    o = out_pool.tile([B_LOC, NSEG_G * D], FP32, tag="out")
                    if g == GROUPS - 1:
                        _emit_group_tail(nc, t, o[:])
                    else:
                        _emit_group_mix(nc, t, o[:])
                    nc.scalar.dma_start(
                        out=y[:, g * NSEG_G : (g + 1) * NSEG_G, :],
                        in_=o[:].rearrange("b (s d) -> b s d", d=D),
                    )
    nc.finalize()
    return nc


def _sharded_from_nc(nc):
    """Build the 8-way-sharded jitted executable for a finalized Bass module.

    Mirrors bass2jax.run_bass_via_pjrt's multi-core branch (shard_map over a
    'core' mesh; per-device shard == the BIR-declared per-core shape) but
    without output-buffer donation so the same function can be called in a
    timing loop with device-resident inputs.
    """
    import jax
    from jax.experimental.shard_map import shard_map
    from jax.sharding import Mesh, NamedSharding, PartitionSpec

    from concourse import bass2jax, mybir as _mybir

    bass2jax.install_neuronx_cc_hook()

    in_names, out_names, out_avals, zero_outs = [], [], [], []
    partition_name = (
        nc.partition_id_tensor.name if nc.partition_id_tensor else None
    )
    for alloc in nc.m.functions[0].allocations:
        if not isinstance(alloc, _mybir.MemoryLocationSet):
            continue
        name = alloc.memorylocations[0].name
        if alloc.kind == "ExternalInput":
            if name != partition_name:
                in_names.append(name)
        elif alloc.kind == "ExternalOutput":
            shape = tuple(alloc.tensor_shape)
            dtype = _mybir.dt.np(alloc.dtype)
            out_names.append(name)
            out_avals.append(jax.core.ShapedArray(shape, dtype))
            zero_outs.append(np.zeros(shape, dtype))
    n_params = len(in_names)
    all_in_names = list(in_names) + list(out_names)
    if partition_name is not None:
        all_in_names.append(partition_name)

    def _body(*args):
        operands = list(args)
        if partition_name is not None:
            operands.append(bass2jax.partition_id_tensor())
        outs = bass2jax._bass_exec_p.bind(
            *operands,
            out_avals=tuple(out_avals),
            in_names=tuple(all_in_names),
            out_names=tuple(out_names),
            lowering_input_output_aliases=(),
            sim_require_finite=True,
            sim_require_nnan=True,
            nc=nc,
        )
        return tuple(outs)

    devices = jax.devices()[:N_CORES]
    mesh = Mesh(np.asarray(devices), ("core",))
    n_outs = len(out_names)
    in_specs = (PartitionSpec("core"),) * (n_params + n_outs)
    out_specs = (PartitionSpec("core"),) * n_outs
    sharded = jax.jit(
        shard_map(
            _body, mesh=mesh, in_specs=in_specs, out_specs=out_specs,
            check_rep=False,
        ),
        keep_unused=True,
    )
    in_sharding = NamedSharding(mesh, PartitionSpec("core"))
    return sharded, zero_outs, in_sharding


@lru_cache(maxsize=8)
def _compiled(reps: int = 1):
    return _sharded_from_nc(_build(reps))


def _put_inputs(emb_vector: np.ndarray, reps: int = 1):
    import jax

    sharded, zero_outs, in_sharding = _compiled(reps)
    x = np.ascontiguousarray(emb_vector, dtype=np.float32)
    dx = jax.device_put(x, in_sharding)
    dzeros = [
        jax.device_put(
            np.zeros((N_CORES * z.shape[0], *z.shape[1:]), z.dtype), in_sharding
        )
        for z in zero_outs
    ]
    return sharded, dx, dzeros


def kernel(emb_vector: np.ndarray) -> np.ndarray:
    sharded, dx, dzeros = _put_inputs(emb_vector)
    (out,) = sharded(dx, *dzeros)
    return np.asarray(out)


def bench(emb_vector: np.ndarray, iters: int = 30, warmup: int = 5,
          reps: int = 1):
    """Steady-state per-call wall time of the sharded executable, ns."""
    import time

    sharded, dx, dzeros = _put_inputs(emb_vector, reps)
    for _ in range(warmup):
        (out,) = sharded(dx, *dzeros)
    out.block_until_ready()
    t0 = time.perf_counter()
    for _ in range(iters):
        (out,) = sharded(dx, *dzeros)
    out.block_until_ready()
    t1 = time.perf_counter()
    return (t1 - t0) / iters * 1e9, np.asarray(out)


def measure_exec_ns(emb_vector: np.ndarray, lo: int = 2, hi: int = 22,
                    iters: int = 20, n_pairs: int = 9):
    """Marginal per-execution HW time via in-NEFF workload repetition:
    (t(hi reps) - t(lo reps)) / (hi - lo) cancels per-dispatch client/RPC
    overhead and NEFF preamble/postamble. hi/lo timing loops are
    interleaved (median of per-pair diffs) so device-load drift cancels;
    deep 20-dispatch bursts keep the dispatch pipeline full so RPC latency
    amortizes out."""
    import time

    sharded_hi, dx, dz_hi = _put_inputs(emb_vector, hi)
    sharded_lo, _, dz_lo = _put_inputs(emb_vector, lo)
    for _ in range(4):
        (out,) = sharded_hi(dx, *dz_hi)
        (out_lo,) = sharded_lo(dx, *dz_lo)
    out.block_until_ready()
    out_lo.block_until_ready()
    diffs = []
    for _ in range(n_pairs):
        t0 = time.perf_counter()
        for _ in range(iters):
            (out,) = sharded_hi(dx, *dz_hi)
        out.block_until_ready()
        t1 = time.perf_counter()
        for _ in range(iters):
            (out_lo,) = sharded_lo(dx, *dz_lo)
        out_lo.block_until_ready()
        t2 = time.perf_counter()
        diffs.append(((t1 - t0) - (t2 - t1)) / iters * 1e9)
    med = sorted(diffs)[len(diffs) // 2]
    return med / (hi - lo), np.asarray(out)


# revision 5
# speedup vs baseline: 1.4551x; 1.0007x over previous
"""Segment-mean pooling kernel for Trainium2 (8 NeuronCores, data-parallel).

Input : emb_vector [1024, 2048, 64] f32
Output: [1024, 32, 64] f32 — mean over 32 ragged field segments
        (sizes [32, 64, 96, 64] * 8, summing to 2048).

Sharding: batch axis 0 split across 8 cores (128 rows each). Per core the
128 batch rows sit on the 128 SBUF partitions; fields*embed is the free
axis. The segment pattern repeats every 256 fields, so each core streams 8
groups of [128, 256*64] f32 (64 KiB/partition, contiguous in DRAM).

DMA structure (the lever that matters): each group loads as FOUR 2-MiB
dma_starts into one 64-KiB/partition tile, with a 3-deep tile ring — up
to twelve 2-MiB input DMAs (24 MiB) queued on the SP HWDGE ring at all
times. Chip HBM bandwidth is dynamically shared across the 8 NeuronCores,
so a core only sustains its full share when its DMA queue never drains;
the shallow 2-deep/8-MiB schedule measured 250-260 us/rep marginal while
this one measures ~190 us/rep median (quiet-window min ~162 us ~= the
chip-aggregate HBM roofline; device-sharing bursts inflate the median).
A/B evidence (interleaved hi/lo-reps marginal diffs, 16-24 pairs): deeper
+ finer beats wider on every statistic; 2-MiB chunks beat both 4-MiB and
1-MiB; 3-deep beats 2-deep; single-ring input loads beat alternating
sync/scalar rings; per-128-field chunk tiles with in-tile GPS merges
(q6 family) regress badly despite equal in-flight bytes.

Per group ('mix_sr' split, chosen to keep both compute engines off each
other's SBUF ports): DVE reduces segments 0-2 straight off the raw tile
with strided XY-reduces (1 read port, no intermediate writes) while GPSIMD
folds segment 3's two 32-field blocks with contiguous in-place pairwise
tensor_adds; the last group instead uses a 6/2 DVE/pool balanced fold so
the kernel tail isn't gated by one engine chain. Scale-by-1/size and the
output DMA issue from the ACT engine so the SP sequencer's HWDGE ring only
ever streams input loads. Compute totals (DVE ~155 us, pool ~66 us per
pass) sit well under the DMA span, so the schedule stays DMA-bound.
"""

import os
import sys
from functools import lru_cache

import numpy as np

for _p in ("/opt/trn_rl_repo", os.path.expanduser("~/.axon_site/_ro/trn_rl_repo")):
    if os.path.isdir(_p) and _p not in sys.path:
        sys.path.insert(0, _p)

import concourse.bass as bass
import concourse.bacc as bacc
import concourse.mybir as mybir
from concourse import tile

N_CORES = 8
BATCH, FIELDS, D = 1024, 2048, 64
B_LOC = BATCH // N_CORES          # 128 batch rows per core = SBUF partitions
GROUP_F = 256                     # fields per repeating segment group
GROUPS = FIELDS // GROUP_F        # 8
SEG_SZ = (32, 64, 96, 64)         # segment sizes (fields)
SEG_BLOCKS = ((0, 1), (1, 3), (3, 6), (6, 8))  # 32-field blocks per segment
NSEG_G = 4                        # segments per group
NSEG = NSEG_G * GROUPS            # 32
FP32 = mybir.dt.float32
BLK = 32 * D                      # one 32-field block: 2048 elems


def _emit_group_mix(nc, t, o):
    """Segment means for one group tile t [128, 256*64] into o [128, 4*64].

    DVE strided XY-reduces segments 0-2 off the raw tile (single read
    port); GPSIMD folds segment 3's two blocks pairwise in place.
    """
    t4 = t[:].rearrange("b (k f d) -> b d k f", k=8, d=D)
    for si in range(3):
        k0, k1 = SEG_BLOCKS[si]
        osl = o[:, si * D : (si + 1) * D]
        nc.vector.reduce_sum(
            out=osl, in_=t4[:, :, k0:k1, :], axis=mybir.AxisListType.XY
        )
        nc.scalar.mul(out=osl, in_=osl, mul=1.0 / SEG_SZ[si])
    for width in (1024, 512, 256, 128, 64):
        v = t[:].rearrange("b (k w) -> b k w", w=BLK)
        nc.gpsimd.tensor_add(
            v[:, 6:, :width], v[:, 6:, :width], v[:, 6:, width : 2 * width]
        )
    o3 = o[:, 3 * D : 4 * D]
    nc.gpsimd.tensor_add(
        o3, t[:, 6 * BLK : 6 * BLK + D], t[:, 7 * BLK : 7 * BLK + D]
    )
    nc.gpsimd.tensor_scalar_mul(o3, o3, 1.0 / SEG_SZ[3])


def _emit_group_tail(nc, t, o):
    """Last group: 6/2 DVE/pool pairwise fold so the tail isn't gated by
    one slow engine chain (DVE folds blocks 0-5, GPSIMD blocks 6-7)."""
    nk = 6
    for width in (1024, 512, 256, 128, 64):
        v = t[:].rearrange("b (k w) -> b k w", w=BLK)
        nc.vector.tensor_add(
            v[:, :nk, :width], v[:, :nk, :width], v[:, :nk, width : 2 * width]
        )
        nc.gpsimd.tensor_add(
            v[:, nk:, :width], v[:, nk:, :width], v[:, nk:, width : 2 * width]
        )
    o3 = o[:, 3 * D : 4 * D]
    nc.gpsimd.tensor_add(
        o3, t[:, 6 * BLK : 6 * BLK + D], t[:, 7 * BLK : 7 * BLK + D]
    )
    nc.gpsimd.tensor_scalar_mul(o3, o3, 1.0 / SEG_SZ[3])
    blocks = t[:].rearrange("b (k w) -> b w k", w=BLK)[:, :D, :]
    for si in range(3):
        k0, k1 = SEG_BLOCKS[si]
        osl = o[:, si * D : (si + 1) * D]
        if k1 - k0 == 1:
            nc.scalar.activation(
                out=osl,
                in_=t[:, k0 * BLK : k0 * BLK + D],
                func=mybir.ActivationFunctionType.Copy,
                scale=1.0 / SEG_SZ[si],
            )
        else:
            nc.vector.reduce_sum(
                out=osl, in_=blocks[:, :, k0:k1], axis=mybir.AxisListType.X
            )
            nc.scalar.mul(out=osl, in_=osl, mul=1.0 / SEG_SZ[si])


@lru_cache(maxsize=16)
def _build(reps: int = 1):
    """reps>1 repeats the whole workload back-to-back inside one NEFF —
    used only for timing (marginal per-rep time cancels dispatch+preamble
    overheads)."""
    nc = bacc.Bacc(
        "TRN2", target_bir_lowering=False, debug=False, num_devices=N_CORES
    )
    x = nc.declare_dram_parameter("x", [B_LOC, FIELDS, D], FP32, isOutput=False)
    y = nc.declare_dram_parameter("y", [B_LOC, NSEG, D], FP32, isOutput=True)
    xf = x.rearrange("b f d -> b (f d)")

    GW = GROUP_F * D      # group width in elems (64 KiB/partition)
    QW = GW // 4          # quarter-group: one 2-MiB DMA

    with tile.TileContext(nc) as tc:
        with (
            tc.tile_pool(name="inp", bufs=3) as inp_pool,
            tc.tile_pool(name="outp", bufs=3) as out_pool,
        ):
            for _ in range(reps):
                for g in range(GROUPS):
                    t = inp_pool.tile([B_LOC, GW], FP32, tag="in")
                    for h in range(4):
                        nc.sync.dma_start(
                            out=t[:, h * QW : (h + 1) * QW],
                            in_=xf[:, g * GW + h * QW : g * GW + (h + 1) * QW],
                        )
                    o = out_pool.tile([B_LOC, NSEG_G * D], FP32, tag="out")
                    if g == GROUPS - 1:
                        _emit_group_tail(nc, t, o[:])
                    else:
                        _emit_group_mix(nc, t, o[:])
                    nc.scalar.dma_start(
                        out=y[:, g * NSEG_G : (g + 1) * NSEG_G, :],
                        in_=o[:].rearrange("b (s d) -> b s d", d=D),
                    )
    nc.finalize()
    return nc


def _sharded_from_nc(nc):
    """Build the 8-way-sharded jitted executable for a finalized Bass module.

    Mirrors bass2jax.run_bass_via_pjrt's multi-core branch (shard_map over a
    'core' mesh; per-device shard == the BIR-declared per-core shape) but
    without output-buffer donation so the same function can be called in a
    timing loop with device-resident inputs.
    """
    import jax
    from jax.experimental.shard_map import shard_map
    from jax.sharding import Mesh, NamedSharding, PartitionSpec

    from concourse import bass2jax, mybir as _mybir

    bass2jax.install_neuronx_cc_hook()

    in_names, out_names, out_avals, zero_outs = [], [], [], []
    partition_name = (
        nc.partition_id_tensor.name if nc.partition_id_tensor else None
    )
    for alloc in nc.m.functions[0].allocations:
        if not isinstance(alloc, _mybir.MemoryLocationSet):
            continue
        name = alloc.memorylocations[0].name
        if alloc.kind == "ExternalInput":
            if name != partition_name:
                in_names.append(name)
        elif alloc.kind == "ExternalOutput":
            shape = tuple(alloc.tensor_shape)
            dtype = _mybir.dt.np(alloc.dtype)
            out_names.append(name)
            out_avals.append(jax.core.ShapedArray(shape, dtype))
            zero_outs.append(np.zeros(shape, dtype))
    n_params = len(in_names)
    all_in_names = list(in_names) + list(out_names)
    if partition_name is not None:
        all_in_names.append(partition_name)

    def _body(*args):
        operands = list(args)
        if partition_name is not None:
            operands.append(bass2jax.partition_id_tensor())
        outs = bass2jax._bass_exec_p.bind(
            *operands,
            out_avals=tuple(out_avals),
            in_names=tuple(all_in_names),
            out_names=tuple(out_names),
            lowering_input_output_aliases=(),
            sim_require_finite=True,
            sim_require_nnan=True,
            nc=nc,
        )
        return tuple(outs)

    devices = jax.devices()[:N_CORES]
    mesh = Mesh(np.asarray(devices), ("core",))
    n_outs = len(out_names)
    in_specs = (PartitionSpec("core"),) * (n_params + n_outs)
    out_specs = (PartitionSpec("core"),) * n_outs
    sharded = jax.jit(
        shard_map(
            _body, mesh=mesh, in_specs=in_specs, out_specs=out_specs,
            check_rep=False,
        ),
        keep_unused=True,
    )
    in_sharding = NamedSharding(mesh, PartitionSpec("core"))
    return sharded, zero_outs, in_sharding


@lru_cache(maxsize=8)
def _compiled(reps: int = 1):
    return _sharded_from_nc(_build(reps))


def _put_inputs(emb_vector: np.ndarray, reps: int = 1):
    import jax

    sharded, zero_outs, in_sharding = _compiled(reps)
    x = np.ascontiguousarray(emb_vector, dtype=np.float32)
    dx = jax.device_put(x, in_sharding)
    dzeros = [
        jax.device_put(
            np.zeros((N_CORES * z.shape[0], *z.shape[1:]), z.dtype), in_sharding
        )
        for z in zero_outs
    ]
    return sharded, dx, dzeros


def kernel(emb_vector: np.ndarray) -> np.ndarray:
    sharded, dx, dzeros = _put_inputs(emb_vector)
    (out,) = sharded(dx, *dzeros)
    return np.asarray(out)


def bench(emb_vector: np.ndarray, iters: int = 30, warmup: int = 5,
          reps: int = 1):
    """Steady-state per-call wall time of the sharded executable, ns."""
    import time

    sharded, dx, dzeros = _put_inputs(emb_vector, reps)
    for _ in range(warmup):
        (out,) = sharded(dx, *dzeros)
    out.block_until_ready()
    t0 = time.perf_counter()
    for _ in range(iters):
        (out,) = sharded(dx, *dzeros)
    out.block_until_ready()
    t1 = time.perf_counter()
    return (t1 - t0) / iters * 1e9, np.asarray(out)


def measure_exec_ns(emb_vector: np.ndarray, lo: int = 2, hi: int = 22,
                    iters: int = 20, n_pairs: int = 13):
    """Marginal per-execution HW time via in-NEFF workload repetition:
    (t(hi reps) - t(lo reps)) / (hi - lo) cancels per-dispatch client/RPC
    overhead and NEFF preamble/postamble. hi/lo timing loops are
    interleaved (median of per-pair diffs) so device-load drift cancels;
    deep 20-dispatch bursts keep the dispatch pipeline full so RPC latency
    amortizes out."""
    import time

    sharded_hi, dx, dz_hi = _put_inputs(emb_vector, hi)
    sharded_lo, _, dz_lo = _put_inputs(emb_vector, lo)
    for _ in range(4):
        (out,) = sharded_hi(dx, *dz_hi)
        (out_lo,) = sharded_lo(dx, *dz_lo)
    out.block_until_ready()
    out_lo.block_until_ready()
    diffs = []
    for _ in range(n_pairs):
        t0 = time.perf_counter()
        for _ in range(iters):
            (out,) = sharded_hi(dx, *dz_hi)
        out.block_until_ready()
        t1 = time.perf_counter()
        for _ in range(iters):
            (out_lo,) = sharded_lo(dx, *dz_lo)
        out_lo.block_until_ready()
        t2 = time.perf_counter()
        diffs.append(((t1 - t0) - (t2 - t1)) / iters * 1e9)
    med = sorted(diffs)[len(diffs) // 2]
    return med / (hi - lo), np.asarray(out)


# revision 10
# speedup vs baseline: 1.4906x; 1.0244x over previous
"""Segment-mean pooling kernel for Trainium2 (8 NeuronCores, data-parallel).

Input : emb_vector [1024, 2048, 64] f32
Output: [1024, 32, 64] f32 — mean over 32 ragged field segments
        (sizes [32, 64, 96, 64] * 8, summing to 2048).

Sharding: batch axis 0 split across 8 cores (128 rows each). Per core the
128 batch rows sit on the 128 SBUF partitions; fields*embed is the free
axis. The segment pattern repeats every 256 fields = four 64-field
quarters, so each core streams 32 quarter tiles of [128, 64*64] f32
(16 KiB/partition, 2 MiB per DMA, contiguous in DRAM).

DMA structure (the lever that matters): a 12-deep ring of independent
16-KiB/partition quarter tiles, one 2-MiB `nc.sync.dma_start` each — up to
twelve 2-MiB input DMAs (24 MiB) queued on the SP HWDGE ring at all times,
and each tile recycles as soon as its own readers finish (quarter-granular
WAR handoff). Chip HBM bandwidth is dynamically shared across the 8
NeuronCores, so a core only sustains its full share when its DMA queue
never drains; the original 2-deep/8-MiB schedule measured 250-260 us/rep
marginal while this one measures ~191 us/rep median (quiet-window min
~162 us ~= the chip-aggregate HBM roofline; device-sharing bursts inflate
the median). A/B evidence (interleaved hi/lo-reps marginal diffs, 16-26
pairs per round): in-flight depth dominates everything else; 2-MiB chunks
beat 4-MiB and 1-MiB; 12 independent quarter tiles edge out 3 group tiles
x 4 sub-DMAs by ~3-6 us; alternating sync/scalar input rings, SWDGE out,
batched out-DMAs, and DVE/GPS rebalances are all neutral or worse.

Per-quarter compute (DVE ~19 us/group strided 1-port reduces, GPS ~9 us,
both well under the ~24 us/group DMA span; engines keep off each other's
shared SBUF port pair):
  q0: seg0 = DVE XY-reduce of block0, scale on ACT; seg1 partial (block1).
  q1: seg1 rest -> tmp, GPS-add + ACT scale; seg2 partial (block1).
  q2: seg2 rest (2 blocks) -> tmp, GPS-add + ACT scale.
  q3: seg3 = GPS in-place merge + 5-level pairwise fold + scale.
Output DMAs issue per group from the ACT engine so the SP ring only ever
streams input loads.
"""

import os
import sys
from functools import lru_cache

import numpy as np

for _p in ("/opt/trn_rl_repo", os.path.expanduser("~/.axon_site/_ro/trn_rl_repo")):
    if os.path.isdir(_p) and _p not in sys.path:
        sys.path.insert(0, _p)

import concourse.bass as bass
import concourse.bacc as bacc
import concourse.mybir as mybir
from concourse import tile

N_CORES = 8
BATCH, FIELDS, D = 1024, 2048, 64
B_LOC = BATCH // N_CORES          # 128 batch rows per core = SBUF partitions
GROUP_F = 256                     # fields per repeating segment group
GROUPS = FIELDS // GROUP_F        # 8
SEG_SZ = (32, 64, 96, 64)         # segment sizes (fields)
NSEG_G = 4                        # segments per group
NSEG = NSEG_G * GROUPS            # 32
FP32 = mybir.dt.float32
BLK = 32 * D                      # one 32-field block: 2048 elems
QF = 64 * D                       # one 64-field quarter: 4096 elems


def _emit_quarter(nc, tq, o, tmp_a, tmp_b, h):
    """Compute for quarter h of a group (tile tq [128, 64*64]).
    h=0: seg0(f0:32) seg1a(32:64) | h=1: seg1b(64:96) seg2a(96:128)
    h=2: seg2b(128:192)           | h=3: seg3(192:256)
    """
    t4 = tq[:].rearrange("b (k f d) -> b d k f", k=2, d=D)  # 2 32-field blocks
    if h == 0:
        o0 = o[:, 0:D]
        nc.vector.reduce_sum(out=o0, in_=t4[:, :, 0:1, :], axis=mybir.AxisListType.XY)
        nc.scalar.mul(out=o0, in_=o0, mul=1.0 / SEG_SZ[0])
        o1 = o[:, D : 2 * D]
        nc.vector.reduce_sum(out=o1, in_=t4[:, :, 1:2, :], axis=mybir.AxisListType.XY)
    elif h == 1:
        o1 = o[:, D : 2 * D]
        nc.vector.reduce_sum(out=tmp_a, in_=t4[:, :, 0:1, :], axis=mybir.AxisListType.XY)
        nc.gpsimd.tensor_add(o1, o1, tmp_a)
        nc.scalar.mul(out=o1, in_=o1, mul=1.0 / SEG_SZ[1])
        o2 = o[:, 2 * D : 3 * D]
        nc.vector.reduce_sum(out=o2, in_=t4[:, :, 1:2, :], axis=mybir.AxisListType.XY)
    elif h == 2:
        o2 = o[:, 2 * D : 3 * D]
        nc.vector.reduce_sum(out=tmp_b, in_=t4[:, :, 0:2, :], axis=mybir.AxisListType.XY)
        nc.gpsimd.tensor_add(o2, o2, tmp_b)
        nc.scalar.mul(out=o2, in_=o2, mul=1.0 / SEG_SZ[2])
    else:
        # seg3: GPS merge block1 into block0, then 5-level pairwise fold
        v = tq[:].rearrange("b (k w) -> b k w", w=BLK)
        nc.gpsimd.tensor_add(v[:, 0:1, :], v[:, 0:1, :], v[:, 1:2, :])
        for width in (1024, 512, 256, 128, 64):
            nc.gpsimd.tensor_add(
                v[:, 0, :width], v[:, 0, :width], v[:, 0, width : 2 * width]
            )
        o3 = o[:, 3 * D : 4 * D]
        nc.gpsimd.tensor_scalar_mul(o3, tq[:, 0:D], 1.0 / SEG_SZ[3])


@lru_cache(maxsize=16)
def _build(reps: int = 1):
    """reps>1 repeats the whole workload back-to-back inside one NEFF —
    used only for timing (marginal per-rep time cancels dispatch+preamble
    overheads)."""
    nc = bacc.Bacc(
        "TRN2", target_bir_lowering=False, debug=False, num_devices=N_CORES
    )
    x = nc.declare_dram_parameter("x", [B_LOC, FIELDS, D], FP32, isOutput=False)
    y = nc.declare_dram_parameter("y", [B_LOC, NSEG, D], FP32, isOutput=True)
    xf = x.rearrange("b f d -> b (f d)")

    with tile.TileContext(nc) as tc:
        with (
            tc.tile_pool(name="inp", bufs=12) as inp_pool,
            tc.tile_pool(name="outp", bufs=2) as out_pool,
            tc.tile_pool(name="tmpp", bufs=2) as tmp_pool,
        ):
            for _ in range(reps):
                for g in range(GROUPS):
                    o = out_pool.tile([B_LOC, NSEG_G * D], FP32, tag="out")
                    ta = tmp_pool.tile([B_LOC, D], FP32, tag="ta")
                    tb = tmp_pool.tile([B_LOC, D], FP32, tag="tb")
                    for h in range(4):
                        tq = inp_pool.tile([B_LOC, QF], FP32, tag="q")
                        nc.sync.dma_start(
                            out=tq[:],
                            in_=xf[:, (4 * g + h) * QF : (4 * g + h + 1) * QF],
                        )
                        _emit_quarter(nc, tq, o[:], ta[:], tb[:], h)
                    nc.scalar.dma_start(
                        out=y[:, g * NSEG_G : (g + 1) * NSEG_G, :],
                        in_=o[:].rearrange("b (s d) -> b s d", d=D),
                    )
    nc.finalize()
    return nc


def _sharded_from_nc(nc):
    """Build the 8-way-sharded jitted executable for a finalized Bass module.

    Mirrors bass2jax.run_bass_via_pjrt's multi-core branch (shard_map over a
    'core' mesh; per-device shard == the BIR-declared per-core shape) but
    without output-buffer donation so the same function can be called in a
    timing loop with device-resident inputs.
    """
    import jax
    from jax.experimental.shard_map import shard_map
    from jax.sharding import Mesh, NamedSharding, PartitionSpec

    from concourse import bass2jax, mybir as _mybir

    bass2jax.install_neuronx_cc_hook()

    in_names, out_names, out_avals, zero_outs = [], [], [], []
    partition_name = (
        nc.partition_id_tensor.name if nc.partition_id_tensor else None
    )
    for alloc in nc.m.functions[0].allocations:
        if not isinstance(alloc, _mybir.MemoryLocationSet):
            continue
        name = alloc.memorylocations[0].name
        if alloc.kind == "ExternalInput":
            if name != partition_name:
                in_names.append(name)
        elif alloc.kind == "ExternalOutput":
            shape = tuple(alloc.tensor_shape)
            dtype = _mybir.dt.np(alloc.dtype)
            out_names.append(name)
            out_avals.append(jax.core.ShapedArray(shape, dtype))
            zero_outs.append(np.zeros(shape, dtype))
    n_params = len(in_names)
    all_in_names = list(in_names) + list(out_names)
    if partition_name is not None:
        all_in_names.append(partition_name)

    def _body(*args):
        operands = list(args)
        if partition_name is not None:
            operands.append(bass2jax.partition_id_tensor())
        outs = bass2jax._bass_exec_p.bind(
            *operands,
            out_avals=tuple(out_avals),
            in_names=tuple(all_in_names),
            out_names=tuple(out_names),
            lowering_input_output_aliases=(),
            sim_require_finite=True,
            sim_require_nnan=True,
            nc=nc,
        )
        return tuple(outs)

    devices = jax.devices()[:N_CORES]
    mesh = Mesh(np.asarray(devices), ("core",))
    n_outs = len(out_names)
    in_specs = (PartitionSpec("core"),) * (n_params + n_outs)
    out_specs = (PartitionSpec("core"),) * n_outs
    sharded = jax.jit(
        shard_map(
            _body, mesh=mesh, in_specs=in_specs, out_specs=out_specs,
            check_rep=False,
        ),
        keep_unused=True,
    )
    in_sharding = NamedSharding(mesh, PartitionSpec("core"))
    return sharded, zero_outs, in_sharding


@lru_cache(maxsize=8)
def _compiled(reps: int = 1):
    return _sharded_from_nc(_build(reps))


def _put_inputs(emb_vector: np.ndarray, reps: int = 1):
    import jax

    sharded, zero_outs, in_sharding = _compiled(reps)
    x = np.ascontiguousarray(emb_vector, dtype=np.float32)
    dx = jax.device_put(x, in_sharding)
    dzeros = [
        jax.device_put(
            np.zeros((N_CORES * z.shape[0], *z.shape[1:]), z.dtype), in_sharding
        )
        for z in zero_outs
    ]
    return sharded, dx, dzeros


def kernel(emb_vector: np.ndarray) -> np.ndarray:
    sharded, dx, dzeros = _put_inputs(emb_vector)
    (out,) = sharded(dx, *dzeros)
    return np.asarray(out)


def bench(emb_vector: np.ndarray, iters: int = 30, warmup: int = 5,
          reps: int = 1):
    """Steady-state per-call wall time of the sharded executable, ns."""
    import time

    sharded, dx, dzeros = _put_inputs(emb_vector, reps)
    for _ in range(warmup):
        (out,) = sharded(dx, *dzeros)
    out.block_until_ready()
    t0 = time.perf_counter()
    for _ in range(iters):
        (out,) = sharded(dx, *dzeros)
    out.block_until_ready()
    t1 = time.perf_counter()
    return (t1 - t0) / iters * 1e9, np.asarray(out)


def measure_exec_ns(emb_vector: np.ndarray, lo: int = 2, hi: int = 22,
                    iters: int = 20, n_pairs: int = 13):
    """Marginal per-execution HW time via in-NEFF workload repetition:
    (t(hi reps) - t(lo reps)) / (hi - lo) cancels per-dispatch client/RPC
    overhead and NEFF preamble/postamble. hi/lo timing loops are
    interleaved (median of per-pair diffs) so device-load drift cancels;
    deep 20-dispatch bursts keep the dispatch pipeline full so RPC latency
    amortizes out."""
    import time

    sharded_hi, dx, dz_hi = _put_inputs(emb_vector, hi)
    sharded_lo, _, dz_lo = _put_inputs(emb_vector, lo)
    for _ in range(4):
        (out,) = sharded_hi(dx, *dz_hi)
        (out_lo,) = sharded_lo(dx, *dz_lo)
    out.block_until_ready()
    out_lo.block_until_ready()
    diffs = []
    for _ in range(n_pairs):
        t0 = time.perf_counter()
        for _ in range(iters):
            (out,) = sharded_hi(dx, *dz_hi)
        out.block_until_ready()
        t1 = time.perf_counter()
        for _ in range(iters):
            (out_lo,) = sharded_lo(dx, *dz_lo)
        out_lo.block_until_ready()
        t2 = time.perf_counter()
        diffs.append(((t1 - t0) - (t2 - t1)) / iters * 1e9)
    med = sorted(diffs)[len(diffs) // 2]
    return med / (hi - lo), np.asarray(out)
